# revision 14
# baseline (speedup 1.0000x reference)
"""Trainium2 Bass kernel for nn_BasicBlock (sparse conv x2 + BN + ReLU + residual).

Strategy (8 NeuronCores, SPMD):
  - Points sharded across cores (50000/core). Gather table (x, then h1)
    replicated in each core's HBM.
  - Masked neighbors remapped host-side to a dummy all-zero table row, so
    gathered contributions are exactly zero (no mask multiply on device).
  - Per 512-point tile: one indirect DMA gathers 512*28 rows (128B each) in a
    slot layout [128 part = (klane a, point j), 7 kblocks x 16 groups, 32ch].
    DVE StreamTranspose (32x32 blocks) flips each k-block to channels-on-
    partitions; 7 accumulating PE matmuls (contraction 4k x 32c = 128) with
    host-prepacked W_cat produce out^T [32, 512] in PSUM (float32r, full rate).
  - BN stats as per-tile sum / sum-of-squares partials (pad points contribute
    zero), AllReduce'd across cores; affine+ReLU applied in a streaming pass.
  - h1 shards AllGather'd to rebuild the full gather table for layer 2.
  - Final pass fuses BN2 affine + residual + ReLU.
"""
import numpy as np

import concourse.bacc as bacc
import concourse.bass as bass
import concourse.tile as tile
from concourse import mybir
from concourse.bass_utils import run_bass_kernel_spmd

F32 = mybir.dt.float32
F32R = mybir.dt.float32r
I32 = mybir.dt.int32
AX = mybir.AxisListType
ALU = mybir.AluOpType
ACT = mybir.ActivationFunctionType

N_POINTS = 400000
N_CORES = 8
C = 32          # channels
K = 27          # kernel offsets
KPAD = 28       # padded to 4-lane blocks
NB = KPAD // 4  # 7 contraction blocks of 4 k-lanes
TILE = 512      # points per tile
GRP = TILE // 32  # 16 point-groups per tile
SLOTF = NB * GRP  # 112 gather rows per partition per tile
EPS = 1e-5


def _pbcast(ap2d, parts, mid):
    """[P, C] SBUF tile -> [parts, mid, C] AP broadcasting along a middle dim."""
    return bass.AP(
        tensor=ap2d.tensor,
        offset=ap2d.offset,
        ap=[[ap2d.ap[0][0], parts], [0, mid], ap2d.ap[1]],
    )


def _row_bcast(dram_row, parts):
    """[1, C] DRAM row -> [parts, C] partition-broadcast AP (for DMA)."""
    return bass.AP(
        tensor=dram_row.tensor,
        offset=dram_row.offset,
        ap=[[0, parts], [1, C]],
    )


def build_program(n_points=N_POINTS, n_cores=N_CORES, repeat=1, debug_out=False):
    n_shard = n_points // n_cores
    assert n_points % n_cores == 0
    ntiles = (n_shard + TILE - 1) // TILE
    n_shard_pad = ntiles * TILE
    n_tbl1 = n_points + 8           # x table + 8 dummy zero rows
    # layer-2 table: AllGather of per-core [n_shard + 1] shards (last row of
    # each shard is a zero row used for masked edges)
    n_loc = max(n_shard_pad, n_shard + 1)
    n_tbl2 = n_cores * (n_shard + 1)

    nc = bacc.Bacc("TRN2", target_bir_lowering=False, debug=False,
                   num_devices=n_cores)

    x_pad = nc.dram_tensor("x_pad", [n_tbl1, C], F32, kind="ExternalInput")
    x_res = nc.dram_tensor("x_res", [n_shard, C], F32, kind="ExternalInput")
    idx1 = nc.dram_tensor("idx1", [ntiles * 128, SLOTF], I32, kind="ExternalInput")
    idx2 = nc.dram_tensor("idx2", [ntiles * 128, SLOTF], I32, kind="ExternalInput")
    w1c = nc.dram_tensor("w1c", [NB, 128, C], F32, kind="ExternalInput")
    w2c = nc.dram_tensor("w2c", [NB, 128, C], F32, kind="ExternalInput")
    gam1 = nc.dram_tensor("gam1", [C, 1], F32, kind="ExternalInput")
    bet1 = nc.dram_tensor("bet1", [C, 1], F32, kind="ExternalInput")
    gam2 = nc.dram_tensor("gam2", [C, 1], F32, kind="ExternalInput")
    bet2 = nc.dram_tensor("bet2", [C, 1], F32, kind="ExternalInput")
    out_ext = nc.dram_tensor("out", [n_shard, C], F32, kind="ExternalOutput")
    if debug_out:
        d_out1 = nc.dram_tensor("d_out1", [n_shard_pad, C], F32, kind="ExternalOutput")
        d_h1f = nc.dram_tensor("d_h1f", [n_cores * (n_shard + 1), C], F32,
                               kind="ExternalOutput")
        d_st = nc.dram_tensor("d_st", [C, 4], F32, kind="ExternalOutput")

    groups = [list(range(n_cores))]
    inv_n = 1.0 / float(n_points)

    with tile.TileContext(nc) as tc:
        with (
            tc.tile_pool(name="dpool", bufs=1, space="DRAM") as dpool,
            tc.tile_pool(name="spool", bufs=1) as spool,
            tc.tile_pool(name="gpool", bufs=3) as gpool,
            tc.tile_pool(name="rpool", bufs=9) as rpool,
            tc.tile_pool(name="ipool", bufs=3) as ipool,
            tc.tile_pool(name="wpool", bufs=3) as wpool,
            tc.tile_pool(name="cpool", bufs=4) as cpool,
            tc.tile_pool(name="stpool", bufs=1) as stpool,
            tc.tile_pool(name="ppool", bufs=4, space="PSUM") as ppool,
        ):
            # ---- persistent DRAM intermediates ----
            out1_raw = dpool.tile([n_shard_pad, C], F32)
            h1_local = dpool.tile([n_loc, C], F32)
            h1_full = dpool.tile([n_tbl2, C], F32, addr_space="Shared")
            out2_raw = dpool.tile([n_shard_pad, C], F32)
            st1_in = dpool.tile([C, 2], F32)
            st1_out = dpool.tile([C, 2], F32, addr_space="Shared")
            st2_in = dpool.tile([C, 2], F32)
            st2_out = dpool.tile([C, 2], F32, addr_space="Shared")
            sv1_sc = dpool.tile([1, C], F32)
            sv1_sh = dpool.tile([1, C], F32)
            sv2_sc = dpool.tile([1, C], F32)
            sv2_sh = dpool.tile([1, C], F32)

            # ---- one-time loads ----
            w1_sb = spool.tile([128, NB, C], F32)
            nc.sync.dma_start(out=w1_sb, in_=w1c[:].rearrange("b p c -> p b c"))
            w2_sb = spool.tile([128, NB, C], F32)
            nc.sync.dma_start(out=w2_sb, in_=w2c[:].rearrange("b p c -> p b c"))
            g1_sb = spool.tile([C, 1], F32)
            nc.sync.dma_start(out=g1_sb, in_=gam1[:])
            b1_sb = spool.tile([C, 1], F32)
            nc.sync.dma_start(out=b1_sb, in_=bet1[:])
            g2_sb = spool.tile([C, 1], F32)
            nc.sync.dma_start(out=g2_sb, in_=gam2[:])
            b2_sb = spool.tile([C, 1], F32)
            nc.sync.dma_start(out=b2_sb, in_=bet2[:])
            eps_sb = spool.tile([C, 1], F32)
            nc.vector.memset(eps_sb, EPS)
            zrow = spool.tile([1, C], F32)
            nc.vector.memset(zrow, 0.0)

            stats = {}
            for ly in (1, 2):
                a = stpool.tile([C, 2], F32, name=f"sacc{ly}")
                nc.vector.memset(a, 0.0)
                stats[ly] = a

            def conv_pass(idx_dram, table_ap, w_sb, out_raw, layer):
                s_acc = stats[layer]

                def body(iv):
                    it = ipool.tile([128, SLOTF], I32, name="it")
                    nc.sync.dma_start(out=it, in_=idx_dram[bass.ts(iv, 128), :])
                    g = gpool.tile([128, SLOTF, C], F32, name="g")
                    for f in range(SLOTF):
                        nc.gpsimd.indirect_dma_start(
                            out=g[:, f, :],
                            out_offset=None,
                            in_=table_ap,
                            in_offset=bass.IndirectOffsetOnAxis(
                                ap=it[:, f:f + 1], axis=0),
                        )
                    ps = ppool.tile([C, TILE], F32, name="ps")
                    for b in range(NB):
                        r = rpool.tile([128, TILE], F32, name="r")
                        nc.vector.transpose(
                            out=r,
                            in_=g[:, bass.ts(b, GRP), :].rearrange("p a c -> p (a c)"),
                        )
                        nc.tensor.matmul(
                            out=ps,
                            lhsT=w_sb[:, b, :],
                            rhs=r,
                            start=(b == 0),
                            stop=(b == NB - 1),
                        )
                    # BN stat partials accumulated into [C, 2]
                    red = wpool.tile([C, 2], F32, name="red")
                    nc.vector.reduce_sum(out=red[:, 0:1], in_=ps, axis=AX.X)
                    sq = wpool.tile([C, TILE], F32, name="sq")
                    nc.scalar.activation(out=sq, in_=ps, func=ACT.Square,
                                         accum_out=red[:, 1:2])
                    nc.vector.tensor_add(s_acc, s_acc, red)
                    # store rows (transpose back to point-major)
                    st = wpool.tile([C, TILE], F32, name="st")
                    nc.vector.transpose(out=st, in_=ps)
                    nc.sync.dma_start(
                        out=out_raw[bass.ts(iv, TILE), :].rearrange(
                            "(a j) c -> j a c", j=32),
                        in_=st.rearrange("j (a c) -> j a c", a=GRP),
                    )

                with tc.For_i(0, ntiles, 1) as iv:
                    body(iv)

            def bn_affine(layer, st_in, st_out, g_sb, b_sb, sv_sc, sv_sh):
                """AllReduce stats; compute per-channel scale/shift; produce
                [128, C] broadcast tiles via a DRAM roundtrip."""
                tot = stats[layer]
                nc.sync.dma_start(out=st_in[:], in_=tot)
                nc.gpsimd.collective_compute(
                    "AllReduce", ALU.add, replica_groups=groups,
                    ins=[st_in[:]], outs=[st_out[:]],
                )
                gtot = spool.tile([C, 2], F32, name=f"gtot{layer}")
                nc.sync.dma_start(out=gtot, in_=st_out[:])
                mean = spool.tile([C, 1], F32, name=f"mean{layer}")
                nc.vector.tensor_scalar_mul(mean, gtot[:, 0:1], inv_n)
                msq = spool.tile([C, 1], F32, name=f"msq{layer}")
                nc.vector.tensor_scalar_mul(msq, gtot[:, 1:2], inv_n)
                var = spool.tile([C, 1], F32, name=f"var{layer}")
                nc.vector.tensor_mul(var, mean, mean)
                nc.vector.tensor_sub(var, msq, var)
                sd = spool.tile([C, 1], F32, name=f"sd{layer}")
                nc.scalar.activation(out=sd, in_=var, func=ACT.Sqrt,
                                     bias=eps_sb, scale=1.0)
                rstd = spool.tile([C, 1], F32, name=f"rstd{layer}")
                nc.vector.reciprocal(out=rstd, in_=sd)
                scale = spool.tile([C, 1], F32, name=f"scale{layer}")
                nc.vector.tensor_mul(scale, g_sb, rstd)
                shift = spool.tile([C, 1], F32, name=f"shift{layer}")
                nc.vector.tensor_mul(shift, mean, scale)
                nc.vector.tensor_sub(shift, b_sb, shift)
                # [C,1] -> DRAM row -> [128, C] partition-broadcast tiles
                nc.sync.dma_start(out=sv_sc[:], in_=scale)
                nc.sync.dma_start(out=sv_sh[:], in_=shift)
                sc_all = spool.tile([128, C], F32, name=f"sc_all{layer}")
                nc.sync.dma_start(out=sc_all, in_=_row_bcast(sv_sc[:], 128))
                sh_all = spool.tile([128, C], F32, name=f"sh_all{layer}")
                nc.sync.dma_start(out=sh_all, in_=_row_bcast(sv_sh[:], 128))
                return sc_all, sh_all

            # ================= layer 1 =================
            conv_pass(idx1, x_pad[:], w1_sb, out1_raw, 1)
            sc1, sh1 = bn_affine(1, st1_in, st1_out, g1_sb, b1_sb, sv1_sc, sv1_sh)

            # intermezzo: h1 = relu(out1 * scale + shift), 1024 points per tile
            n_imz = n_shard_pad // 1024
            for u in range(n_imz):
                o1 = cpool.tile([128, 8, C], F32, name="o1")
                nc.sync.dma_start(
                    out=o1,
                    in_=out1_raw[bass.ts(u, 1024), :].rearrange("(p r) c -> p r c", r=8))
                t1 = cpool.tile([128, 8, C], F32, name="t1")
                nc.vector.tensor_tensor(out=t1, in0=o1, in1=_pbcast(sc1, 128, 8),
                                        op=ALU.mult)
                nc.vector.tensor_tensor(out=t1, in0=t1, in1=_pbcast(sh1, 128, 8),
                                        op=ALU.add)
                h1t = cpool.tile([128, 8, C], F32, name="h1t")
                nc.scalar.activation(out=h1t, in_=t1, func=ACT.Relu)
                nc.sync.dma_start(
                    out=h1_local[bass.ts(u, 1024), :].rearrange("(p r) c -> p r c", r=8),
                    in_=h1t)

            # zero row at local position n_shard (emitted after the intermezzo
            # so it overwrites the pad-point garbage there)
            nc.sync.dma_start(out=h1_local[n_shard:n_shard + 1, :], in_=zrow)

            # rebuild the full table for layer 2: global table row layout is
            # core-strided: point m lives at (m // n_shard)*(n_shard+1) + m % n_shard
            nc.gpsimd.collective_compute(
                "AllGather", ALU.bypass, replica_groups=groups,
                ins=[h1_local[0:n_shard + 1, :]], outs=[h1_full[0:n_tbl2, :]],
            )

            if debug_out:
                nc.sync.dma_start(out=d_out1[:], in_=out1_raw[:])
                nc.sync.dma_start(out=d_h1f[:], in_=h1_full[:, :])
                dst1 = spool.tile([C, 2], F32, name="dst1")
                nc.sync.dma_start(out=dst1, in_=st1_out[:])
                nc.sync.dma_start(out=d_st[:, 0:2], in_=dst1)

            # ================= layer 2 =================
            conv_pass(idx2, h1_full[:, :], w2_sb, out2_raw, 2)
            sc2, sh2 = bn_affine(2, st2_in, st2_out, g2_sb, b2_sb, sv2_sc, sv2_sh)

            # final: out = relu(out2 * scale2 + shift2 + x)
            nfin = (n_shard + TILE - 1) // TILE
            for u in range(nfin):
                rows = min(TILE, n_shard - u * TILE)
                parts = rows // 4
                assert parts * 4 == rows
                o2 = cpool.tile([128, 4, C], F32, name="o2")
                nc.sync.dma_start(
                    out=o2[:parts],
                    in_=out2_raw[u * TILE:u * TILE + rows, :].rearrange(
                        "(p r) c -> p r c", r=4))
                xr = cpool.tile([128, 4, C], F32, name="xr")
                nc.sync.dma_start(
                    out=xr[:parts],
                    in_=x_res[u * TILE:u * TILE + rows, :].rearrange(
                        "(p r) c -> p r c", r=4))
                tc_ = cpool.tile([128, 4, C], F32, name="tc_")
                nc.vector.tensor_tensor(out=tc_[:parts], in0=o2[:parts],
                                        in1=_pbcast(sc2, parts, 4), op=ALU.mult)
                nc.vector.tensor_tensor(out=tc_[:parts], in0=tc_[:parts],
                                        in1=_pbcast(sh2, parts, 4), op=ALU.add)
                nc.vector.tensor_add(tc_[:parts], tc_[:parts], xr[:parts])
                fin = cpool.tile([128, 4, C], F32, name="fin")
                nc.scalar.activation(out=fin[:parts], in_=tc_[:parts], func=ACT.Relu)
                nc.sync.dma_start(
                    out=out_ext[u * TILE:u * TILE + rows, :].rearrange(
                        "(p r) c -> p r c", r=4),
                    in_=fin[:parts])

    nc.compile()
    return nc


def prep_inputs(x, W1, gamma1, beta1, W2, gamma2, beta2, nbr1, mask1, nbr2, mask2,
                n_points=N_POINTS, n_cores=N_CORES):
    """Host-side preprocessing: dummy-row remap + slot-order index layout."""
    n_shard = n_points // n_cores
    ntiles = (n_shard + TILE - 1) // TILE
    n_shard_pad = ntiles * TILE

    # layer-1 table: x + 8 zero rows; masked edges spread across the 8
    x_pad = np.concatenate([x, np.zeros((8, C), np.float32)], axis=0)

    def pack_w(W):
        Wp = np.concatenate([W, np.zeros((KPAD - K, C, C), np.float32)], axis=0)
        return np.ascontiguousarray(Wp.reshape(NB, 4, C, C).reshape(NB, 4 * C, C))

    w1c = pack_w(np.asarray(W1, np.float32))
    w2c = pack_w(np.asarray(W2, np.float32))

    def remap1(m, valid):
        return np.where(valid, m, n_points + (m & 7)).astype(np.int32)

    def remap2(m, valid):
        # core-strided layer-2 table positions; masked -> owning core's zero row
        q, r = np.divmod(m, n_shard)
        return np.where(valid, q * (n_shard + 1) + r,
                        q * (n_shard + 1) + n_shard).astype(np.int32)

    def pack_idx(nbr, mask, s, remap, dummy_pos):
        eff = remap(nbr, mask)
        sh = eff[s * n_shard:(s + 1) * n_shard]
        if n_shard_pad > n_shard:
            sh = np.concatenate(
                [sh, np.full((n_shard_pad - n_shard, K), dummy_pos, np.int32)], axis=0)
        sh = np.concatenate(
            [sh, np.full((n_shard_pad, KPAD - K), dummy_pos, np.int32)], axis=1)
        # [t, g, j, b, a] -> [t, (a j), (b g)]
        arr = sh.reshape(ntiles, GRP, 32, NB, 4).transpose(0, 4, 2, 3, 1)
        return np.ascontiguousarray(arr.reshape(ntiles * 128, SLOTF))

    col = lambda v: np.asarray(v, np.float32).reshape(C, 1)
    in_maps = []
    for s in range(n_cores):
        in_maps.append({
            "x_pad": x_pad,
            "x_res": np.ascontiguousarray(x[s * n_shard:(s + 1) * n_shard]),
            "idx1": pack_idx(nbr1, mask1, s, remap1, n_points),
            "idx2": pack_idx(nbr2, mask2, s, remap2, n_shard),
            "w1c": w1c, "w2c": w2c,
            "gam1": col(gamma1), "bet1": col(beta1),
            "gam2": col(gamma2), "bet2": col(beta2),
        })
    return in_maps


_PROGRAM_CACHE = {}


def kernel(x, W1, b1, gamma1, beta1, W2, b2, gamma2, beta2,
           nbr1, mask1, nbr2, mask2):
    # b1/b2 are dropped: BN immediately follows each conv, so a per-channel
    # bias shifts the mean and cancels exactly in (h - mean).
    x = np.asarray(x, np.float32)
    key = (N_POINTS, N_CORES)
    if key not in _PROGRAM_CACHE:
        _PROGRAM_CACHE[key] = build_program(N_POINTS, N_CORES)
    nc = _PROGRAM_CACHE[key]
    in_maps = prep_inputs(x, W1, gamma1, beta1, W2, gamma2, beta2,
                          nbr1, mask1, nbr2, mask2)
    res = run_bass_kernel_spmd(nc, in_maps, list(range(N_CORES)))
    return np.concatenate([res.results[s]["out"] for s in range(N_CORES)], axis=0)
